# revision 43
# baseline (speedup 1.0000x reference)
"""MoE MLP (Mixtral-style top-2 routing) on 8 Trainium2 NeuronCores.

Strategy: expert-COLUMN-parallel (F-sharding). The router (tiny: T x H x E)
runs on host in fp32, exactly mirroring the reference math. Tokens are
grouped by expert on host; EVERY core processes EVERY expert's token group,
but only a 512-wide slice of the F dimension (core k owns F rows
[k*512,(k+1)*512) of each expert's Wg/Wu and the matching Wd columns).
Per-core work is therefore identical by construction — no load imbalance,
no padding to the max expert count — and each expert's weight slice is
loaded exactly once (48MB/core total). Each core produces a PARTIAL down
projection (its F-slice's contribution); the host sums the 8 partials in
f64 and applies the top-k combine weights in a weighted scatter-add.

Device layout (per core, everything feature-on-partition, token-on-free):
  hT   [H=1024, TC]  all experts' tokens, grouped+padded, transposed
  WgT  [H, 8*512]    per-expert F-slices of gate weight, transposed
  WuT  [H, 8*512]    up weight, same layout
  WdT  [8*512, H]    down weight slices, transposed
  yT   [H, TC]       partial expert outputs (to be summed across cores)

Program: one weight BLOCK per expert (double-buffered); each expert's
tokens form 1-2 passes (<=1152 resident tokens, h double-buffered via a
256-wide h_pre prefetch tile); per pass a loop over ct tiles. Gate/up
matmuls (f32r, full PE rate) accumulate over H in PSUM; ScalarE applies
silu; VectorE multiplies by the up projection; down matmuls accumulate the
512-wide F-slice in PSUM; VectorE copies to y_sb and gpsimd streams each
finished ct tile back to DRAM. The PE stream runs one ct-tile ahead of the
silu/mul stage to hide their latency.
"""

import numpy as np
import concourse.bass as bass
import concourse.mybir as mybir
from concourse.bass_utils import run_bass_kernel_spmd

f32 = mybir.dt.float32
f32r = mybir.dt.float32r

B, S, H, F, E = 4, 2048, 1024, 4096, 8
ACT_FUNC = mybir.ActivationFunctionType.Silu  # swapped in coresim_check.py
KT = H // 128  # 8 k-tiles of the H contraction
FSH = F // 8  # 512-wide per-core F slice
FT_PER = FSH // 128  # 4 f-tiles per slice
HT = H // 128  # 8 output H tiles
CT_W = 512  # max token tile width (moving dim N)
PASS_MAX = 1152  # SBUF budget for h_sb/y_sb columns


def _split_tiles(pass_size):
    """Split a pass into ct tiles, leading with a 256 tile (small first
    DMA so the PE can start early / h_pre prefetch is cheap), then greedy
    512s with the remainder pushed to the end (so the final tile — and the
    final y store — is as small as possible). All multiples of 128, >=256."""
    widths = [256]
    rest = pass_size - 256
    if rest < 256:
        assert 256 <= pass_size <= 512, pass_size
        return [pass_size]
    while rest:
        if rest <= 512:
            widths.append(rest)
            rest = 0
        elif rest - 512 >= 256:
            widths.append(512)
            rest -= 512
        else:
            widths.append(rest - 256)
            rest = 256
    assert sum(widths) == pass_size and all(256 <= w <= 512 for w in widths), widths
    return widths


def _expert_passes(ce_pad):
    """Split one expert's padded token count into passes of <=PASS_MAX,
    near-equal, multiples of 64."""
    n = -(-ce_pad // PASS_MAX)
    base = (ce_pad // n) // 64 * 64
    out = [base] * n
    rem = (ce_pad - base * n) // 64
    for i in range(rem):
        out[i] += 64
    assert sum(out) == ce_pad and all(256 <= ps <= PASS_MAX for ps in out), out
    return out


def build_program(expert_sizes, repeat=1):
    """Per-core Bass program. `expert_sizes`: padded token count per
    present expert (in block order). Each expert is one weight block
    spanning 1+ token passes. `repeat` re-runs everything (bench only)."""
    NWB = len(expert_sizes)
    pass_sizes = []
    pass_wb = []  # weight block (expert slot) per pass
    for b, ce in enumerate(expert_sizes):
        for ps in _expert_passes(ce):
            pass_sizes.append(ps)
            pass_wb.append(b)
    TC = sum(pass_sizes)
    tok0 = [sum(pass_sizes[:p]) for p in range(len(pass_sizes))]

    pass_sizes = pass_sizes * repeat
    pass_tok0 = tok0 * repeat
    pass_wb = [b + r * NWB for r in range(repeat) for b in pass_wb]
    NP = len(pass_sizes)
    NB = NWB * repeat  # global weight-block sequence length
    PSMAX = max(pass_sizes)
    tiles = [_split_tiles(ps) for ps in pass_sizes]
    NCT = [len(t) for t in tiles]
    tile_offs = [[sum(tiles[p][:i]) for i in range(NCT[p])] for p in range(NP)]

    # ctg enumeration: for p, for ct -> (p, ct, width, offset)
    ctg_base = [0] * (NP + 1)
    for p in range(NP):
        ctg_base[p + 1] = ctg_base[p] + NCT[p]
    TOTAL_CT = ctg_base[NP]
    ctg_pfc = []
    for p in range(NP):
        for ct in range(NCT[p]):
            ctg_pfc.append((p, ct, tiles[p][ct], tile_offs[p][ct]))

    # last ctg (exclusive) of each weight block
    blk_pass_last = {}
    for p in range(NP):
        blk_pass_last[pass_wb[p]] = p
    blk_ctg_end = {b: ctg_base[blk_pass_last[b] + 1] for b in blk_pass_last}
    blk_pass_first = {}
    for p in range(NP - 1, -1, -1):
        blk_pass_first[pass_wb[p]] = p

    SLOTS = max(NCT)

    # Per-tile-SLOT h DMA counts: slot j of pass p has been loaded
    # ht_cnt[p][j] times through pass p. DMA completions within a queue
    # burst are unordered, so each slot gets its OWN semaphore and waits
    # are on full per-slot cumulative totals (race-detector-valid: pass
    # p's slot-j load is gated on a gu that observed pass p-1's count).
    ht_cnt = []
    cnt = [0] * SLOTS
    for p in range(NP):
        for j in range(NCT[p]):
            cnt[j] += 1
        ht_cnt.append(list(cnt))

    # y store counts per slot, same scheme (stores are gated on yupds that
    # observed the previous pass's store counts)
    yd_cnt = []
    cnt = [0] * SLOTS
    for p in range(NP):
        for j in range(NCT[p]):
            cnt[j] += 4 if p == NP - 1 else 1
        yd_cnt.append(list(cnt))

    def _overlaps(p, lo, hi):
        """Tile indices of pass p whose column range intersects [lo, hi)."""
        return [
            i
            for i, (o, w) in enumerate(zip(tile_offs[p], tiles[p]))
            if o < hi and o + w > lo
        ]

    # Weight-block thresholds: block 0 is ft-granular on dedicated sems
    # (s_p[ft] pairs + s_wd); blocks >= 1 alternate parity sems s_w0/s_w1
    # (+48 each). The b-2 double-buffer gate proves all same-parity
    # predecessors complete, so full-block cumulative totals on the parity
    # sem are unambiguous even with blocks b-1 and b in flight.
    def swp_need(bs):
        assert bs >= 1
        n_parity = (bs + 1) // 2 if bs % 2 == 1 else bs // 2
        return 48 * n_parity

    nc = bass.Bass()
    hT = nc.declare_dram_parameter("hT", [H, TC], f32r, isOutput=False)
    wg = nc.declare_dram_parameter("WgT", [H, NWB * FSH], f32r, isOutput=False)
    wu = nc.declare_dram_parameter("WuT", [H, NWB * FSH], f32r, isOutput=False)
    wd = nc.declare_dram_parameter("WdT", [NWB * FSH, H], f32r, isOutput=False)
    yT = nc.declare_dram_parameter("yT", [H, TC], f32, isOutput=True)

    hT_v = hT.rearrange("(k p) t -> p k t", p=128)  # [128, KT, TC]
    wg_v = wg.rearrange("(k p) f -> p k f", p=128)  # [128, KT, NWB*FSH]
    wu_v = wu.rearrange("(k p) f -> p k f", p=128)
    wd_v = wd.rearrange("(q p) h -> p q h", p=128)  # [128, NWB*FT_PER, H]
    yT_v = yT.rearrange("(k p) t -> p k t", p=128)  # [128, HT, TC]

    from contextlib import ExitStack

    with ExitStack() as ctx:
        en = ctx.enter_context
        h_sb = en(nc.sbuf_tensor("h_sb", [128, KT, PSMAX], f32r))
        h_pre = en(nc.sbuf_tensor("h_pre", [128, KT, CT_W], f32r))
        y_sb = en(nc.sbuf_tensor("y_sb", [128, HT, PSMAX], f32))
        wg_sb = en(nc.sbuf_tensor("wg_sb", [128, 2, KT, FSH], f32r))
        wu_sb = en(nc.sbuf_tensor("wu_sb", [128, 2, KT, FSH], f32r))
        wd_sb = en(nc.sbuf_tensor("wd_sb", [128, 2, FT_PER, H], f32r))
        act_sb = en(nc.sbuf_tensor("act_sb", [128, 2, FT_PER, CT_W], f32r))

        g_ps = [en(nc.psum_tensor(f"g_ps{i}", [128, CT_W], f32)) for i in range(2)]
        u_ps = [en(nc.psum_tensor(f"u_ps{i}", [128, CT_W], f32)) for i in range(2)]
        yp_ps = [en(nc.psum_tensor(f"yp_ps{i}", [128, CT_W], f32)) for i in range(4)]

        s_p = [en(nc.semaphore(name=f"s_p{i}")) for i in range(FT_PER)]  # blk0 wg/wu pairs
        s_wd0 = en(nc.semaphore(name="s_wd0"))  # blk0 wd
        s_w0 = en(nc.semaphore(name="s_w0"))  # even blocks >= 2 (48/blk)
        s_w1 = en(nc.semaphore(name="s_w1"))  # odd blocks (48/blk)
        s_ht = [en(nc.semaphore(name=f"s_ht{j}")) for j in range(SLOTS)]  # h tile slots
        s_yd = [en(nc.semaphore(name=f"s_yd{j}")) for j in range(SLOTS)]  # y store slots
        s_g = en(nc.semaphore(name="s_g"))  # PE: gate groups done (1/gi)
        s_u = en(nc.semaphore(name="s_u"))  # PE: up groups done (1/gi)
        s_silu = en(nc.semaphore(name="s_silu"))  # ACT: silu into act done (1/gi)
        s_mul = en(nc.semaphore(name="s_mul"))  # DVE: act *= up done (1/gi)
        s_down = en(nc.semaphore(name="s_down"))  # PE: down groups done (1/di)
        s_yupd = en(nc.semaphore(name="s_yupd"))  # DVE: y copy done (1/di)

        block = en(nc.Block())

        # ---------------- weight DMA stream (sync engine / HWDGE) --------
        @block.sync
        def _(sync):
            for bs in range(NB):
                b = bs % NWB  # slice index into the weight buffers
                buf = bs % 2
                if bs == 1:
                    # block 1 isn't needed until its first pass (~2 passes
                    # in); keep its 6MB out of the contended startup window
                    for j in range(NCT[0]):
                        sync.wait_ge(s_ht[j], 16 * ht_cnt[0][j])
                if bs >= 2:
                    # WAR: buffer bs%2 still read by block bs-2's gus/downs
                    sync.wait_ge(s_down, 8 * blk_ctg_end[bs - 2])
                fsl = slice(b * FSH, (b + 1) * FSH)
                qsl = slice(b * FT_PER, (b + 1) * FT_PER)
                if bs == 0:
                    for ft in range(FT_PER):
                        f0 = b * FSH + ft * 128
                        sync.dma_start(
                            wg_sb[:, buf, :, ft * 128 : (ft + 1) * 128],
                            wg_v[:, :, f0 : f0 + 128],
                        ).then_inc(s_p[ft], 16)
                        sync.dma_start(
                            wu_sb[:, buf, :, ft * 128 : (ft + 1) * 128],
                            wu_v[:, :, f0 : f0 + 128],
                        ).then_inc(s_p[ft], 16)
                    if NCT[0] >= 2:
                        # wd isn't needed until the first down (~2 gu
                        # groups in); let h ct1 take the DMA engines first
                        sync.wait_ge(s_ht[1], 16)
                    sync.dma_start(wd_sb[:, buf], wd_v[:, qsl, :]).then_inc(s_wd0, 16)
                else:
                    sw = s_w1 if bs % 2 == 1 else s_w0
                    sync.dma_start(wg_sb[:, buf], wg_v[:, :, fsl]).then_inc(sw, 16)
                    sync.dma_start(wu_sb[:, buf], wu_v[:, :, fsl]).then_inc(sw, 16)
                    sync.dma_start(wd_sb[:, buf], wd_v[:, qsl, :]).then_inc(sw, 16)

        # ---------------- hT loads + y stores (gpsimd / SWDGE) -----------
        @block.gpsimd
        def _(gp):
            def load_h(p):
                # tile 0 of pass p>=1 goes to the h_pre prefetch buffer,
                # issued as soon as pass p-1's first gu released it
                if p >= 1:
                    gp.wait_ge(s_u, 4 * (ctg_base[p - 1] + 1))
                    w0 = tiles[p][0]
                    tsl = slice(pass_tok0[p], pass_tok0[p] + w0)
                    gp.dma_start(h_pre[:, :, :w0], hT_v[:, :, tsl]).then_inc(
                        s_ht[0], 16
                    )
                off = 0
                for i, wdt in enumerate(tiles[p]):
                    if p >= 1 and i == 0:
                        off += wdt
                        continue
                    if p >= 1:
                        # WAR on h_sb cols [off, off+wdt): last readers are
                        # pass p-1's gus of the overlapping tiles (PE is
                        # in-order, so this also covers all earlier passes;
                        # empty overlap -> m=-1 waits out passes < p-1)
                        m = max(_overlaps(p - 1, off, off + wdt), default=-1)
                        gp.wait_ge(s_u, 4 * (ctg_base[p - 1] + m + 1))
                    tsl = slice(pass_tok0[p] + off, pass_tok0[p] + off + wdt)
                    gp.dma_start(
                        h_sb[:, :, off : off + wdt], hT_v[:, :, tsl]
                    ).then_inc(s_ht[i], 16)
                    off += wdt

            def store_y_tiles(p):
                # stream each finished ct tile out as soon as its yupds
                # land; the last pass goes out in ht-pair pieces so the
                # final stores pipeline with the trailing yupds
                for j in range(NCT[p]):
                    ctg = ctg_base[p] + j
                    coff = tile_offs[p][j]
                    ctw = tiles[p][j]
                    tsl = slice(pass_tok0[p] + coff, pass_tok0[p] + coff + ctw)
                    if p == NP - 1:
                        for hp in range(4):
                            gp.wait_ge(s_yupd, 8 * ctg + 2 * (hp + 1))
                            gp.dma_start(
                                yT_v[:, 2 * hp : 2 * hp + 2, tsl],
                                y_sb[:, 2 * hp : 2 * hp + 2, coff : coff + ctw],
                            ).then_inc(s_yd[j], 16)
                    else:
                        gp.wait_ge(s_yupd, 8 * (ctg + 1))
                        gp.dma_start(
                            yT_v[:, :, tsl], y_sb[:, :, coff : coff + ctw]
                        ).then_inc(s_yd[j], 16)

            load_h(0)
            if NP > 1:
                load_h(1)
            for p in range(2, NP):
                # stores(p-2) gate on pass p-2's yupds, which complete
                # before load_h(p)'s gates (pass p-1 gus) — emit first
                store_y_tiles(p - 2)
                load_h(p)
            for p in range(max(NP - 2, 0), NP):
                store_y_tiles(p)

        # ---------------- PE stream (one ct-tile lookahead) ----------------
        @block.tensor
        def _(te):
            def gu(ctg):
                p, ct, ctw, coff = ctg_pfc[ctg]
                bs = pass_wb[p]
                buf = bs % 2
                te.wait_ge(s_ht[ct], 16 * ht_cnt[p][ct])
                if ct == 0 and bs > 0 and p == blk_pass_first[bs]:
                    te.wait_ge(s_w1 if bs % 2 == 1 else s_w0, swp_need(bs))
                use_pre = p >= 1 and ct == 0
                csl = slice(coff, coff + ctw)
                for ft in range(FT_PER):
                    gi = ctg * 4 + ft
                    gb = gi % 2
                    if p == 0 and ct == 0:
                        te.wait_ge(s_p[ft], 32)
                    if gi >= 2:
                        te.wait_ge(s_silu, gi - 1)
                    for k in range(KT):
                        rhs = h_pre[:, k, :ctw] if use_pre else h_sb[:, k, csl]
                        mm = nc.tensor.matmul(
                            g_ps[gb][:, :ctw],
                            wg_sb[:, buf, k, ft * 128 : (ft + 1) * 128],
                            rhs,
                            start=(k == 0),
                            stop=(k == KT - 1),
                        )
                        if k == KT - 1:
                            mm.then_inc(s_g, 1)
                    if gi >= 2:
                        te.wait_ge(s_mul, gi - 1)
                    for k in range(KT):
                        rhs = h_pre[:, k, :ctw] if use_pre else h_sb[:, k, csl]
                        mm = nc.tensor.matmul(
                            u_ps[gb][:, :ctw],
                            wu_sb[:, buf, k, ft * 128 : (ft + 1) * 128],
                            rhs,
                            start=(k == 0),
                            stop=(k == KT - 1),
                        )
                        if k == KT - 1:
                            mm.then_inc(s_u, 1)

            def down(ctg):
                p, ct, ctw, coff = ctg_pfc[ctg]
                bs = pass_wb[p]
                buf = bs % 2
                ab = ctg % 2
                if ct == 0 and p == blk_pass_first[bs]:
                    if bs == 0:
                        te.wait_ge(s_wd0, 16)
                    else:
                        te.wait_ge(s_w1 if bs % 2 == 1 else s_w0, swp_need(bs))
                te.wait_ge(s_mul, 4 * (ctg + 1))
                for ht in range(HT):
                    di = ctg * 8 + ht
                    db = di % 4
                    if di >= 4:
                        te.wait_ge(s_yupd, di - 3)
                    for ft in range(FT_PER):
                        mm = nc.tensor.matmul(
                            yp_ps[db][:, :ctw],
                            wd_sb[:, buf, ft, ht * 128 : (ht + 1) * 128],
                            act_sb[:, ab, ft, :ctw],
                            start=(ft == 0),
                            stop=(ft == FT_PER - 1),
                        )
                        if ft == FT_PER - 1:
                            mm.then_inc(s_down, 1)

            gu(0)
            for ctg in range(TOTAL_CT):
                if ctg + 1 < TOTAL_CT:
                    gu(ctg + 1)
                down(ctg)

        # ---------------- ACT stream (silu into act tile) ------------------
        @block.scalar
        def _(sc):
            for ctg in range(TOTAL_CT):
                ab = ctg % 2
                ctw = ctg_pfc[ctg][2]
                for ft in range(FT_PER):
                    gi = ctg * 4 + ft
                    gb = gi % 2
                    if ft == 0 and ctg >= 2:
                        # WAR on act_sb[ab]: down mms of ctg-2 done
                        sc.wait_ge(s_down, 8 * (ctg - 1))
                    sc.wait_ge(s_g, gi + 1)
                    nc.scalar.activation(
                        act_sb[:, ab, ft, :ctw],
                        g_ps[gb][:, :ctw],
                        ACT_FUNC,
                    ).then_inc(s_silu, 1)

        # ---------------- DVE stream (mul + y copy) ------------------
        @block.vector
        def _(ve):
            def muls(ctg):
                ab = ctg % 2
                ctw = ctg_pfc[ctg][2]
                for ft in range(FT_PER):
                    gi = ctg * 4 + ft
                    gb = gi % 2
                    ve.wait_ge(s_silu, gi + 1)
                    ve.wait_ge(s_u, gi + 1)
                    nc.vector.tensor_mul(
                        act_sb[:, ab, ft, :ctw],
                        act_sb[:, ab, ft, :ctw],
                        u_ps[gb][:, :ctw],
                    ).then_inc(s_mul, 1)

            def yupd(ctg):
                p, ct, ctw, coff = ctg_pfc[ctg]
                csl = slice(coff, coff + ctw)
                for ht in range(HT):
                    di = ctg * 8 + ht
                    db = di % 4
                    ve.wait_ge(s_down, di + 1)
                    if ht == 0 and p > 0:
                        # WAR on y_sb cols: stores of ALL overlapping tiles
                        # through pass p-1 must have drained (yd_cnt is
                        # cumulative, so this covers earlier passes too);
                        # empty overlap -> wait out every slot
                        ov = _overlaps(p - 1, coff, coff + ctw) or range(SLOTS)
                        for i in ov:
                            ve.wait_ge(s_yd[i], 16 * yd_cnt[p - 1][i])
                    nc.vector.tensor_copy(
                        y_sb[:, ht, csl], yp_ps[db][:, :ctw]
                    ).then_inc(s_yupd, 1)

            muls(0)
            for ctg in range(TOTAL_CT):
                # mirror the PE stream's emission order exactly
                if ctg + 1 < TOTAL_CT:
                    muls(ctg + 1)
                yupd(ctg)

    return nc


# ----------------------------------------------------------------------------
# Host side
# ----------------------------------------------------------------------------


def _route(h, Wr, topk):
    """Exact fp32 replica of the reference router. Returns sel [T,k], w [T,k]."""
    logits = h @ Wr.T  # [T, E]
    logits = logits.astype(np.float32)
    m = logits.max(axis=-1, keepdims=True)
    e = np.exp(logits - m)
    p = e / e.sum(axis=-1, keepdims=True)
    sel = np.argsort(-p, axis=-1, kind="stable")[:, :topk]  # ties -> lower idx
    w = np.take_along_axis(p, sel, axis=-1)
    if topk != 1:
        w = w / w.sum(axis=-1, keepdims=True)
    return sel, w.astype(np.float32)


def kernel(x, Wr, Wg, Wu, Wd, topk):
    topk = int(topk)
    x = np.asarray(x, dtype=np.float32)
    Wr = np.asarray(Wr, dtype=np.float32)
    Wg = np.asarray(Wg, dtype=np.float32)
    Wu = np.asarray(Wu, dtype=np.float32)
    Wd = np.asarray(Wd, dtype=np.float32)

    T = x.shape[0] * x.shape[1]
    h = np.ascontiguousarray(x.reshape(T, H))

    sel, w = _route(h, Wr, topk)

    idx = [None] * E
    wts = [None] * E
    for e in range(E):
        tok, kk = np.nonzero(sel == e)
        idx[e] = tok
        wts[e] = w[tok, kk]
    counts = [len(i) for i in idx]

    present = [e for e in range(E) if counts[e] > 0]
    sizes = [max(256, ((counts[e] + 63) // 64) * 64) for e in present]
    TC = sum(sizes)
    tok0 = [sum(sizes[:i]) for i in range(len(sizes))]

    nc = build_program(sizes)

    # hT: all experts' tokens grouped and padded — identical on every core
    hTfull = h.T  # [H, T] view
    hT = np.zeros((H, TC), dtype=np.float32)
    for i, e in enumerate(present):
        hT[:, tok0[i] : tok0[i] + counts[e]] = hTfull[:, idx[e]]

    # per-core weight slices: core k owns F rows [k*FSH,(k+1)*FSH) of every
    # expert (transposed layouts, concatenated in block order)
    in_maps = []
    for k in range(E):
        fs = slice(k * FSH, (k + 1) * FSH)
        WgT = np.empty((H, len(present) * FSH), dtype=np.float32)
        WuT = np.empty((H, len(present) * FSH), dtype=np.float32)
        WdT = np.empty((len(present) * FSH, H), dtype=np.float32)
        for i, e in enumerate(present):
            WgT[:, i * FSH : (i + 1) * FSH] = Wg[e, fs, :].T
            WuT[:, i * FSH : (i + 1) * FSH] = Wu[e, fs, :].T
            WdT[i * FSH : (i + 1) * FSH, :] = Wd[e, :, fs].T
        in_maps.append({"hT": hT, "WgT": WgT, "WuT": WuT, "WdT": WdT})

    res = run_bass_kernel_spmd(nc, in_maps, core_ids=list(range(E)))

    # sum the 8 partial projections in f64, then combine
    ysum = res.results[0]["yT"].astype(np.float64)
    for k in range(1, E):
        ysum += res.results[k]["yT"]
    out = np.zeros((T, H), dtype=np.float32)
    for i, e in enumerate(present):
        cnt = counts[e]
        ye = ysum[:, tok0[i] : tok0[i] + cnt].T  # [cnt, H] f64
        out[idx[e]] += (wts[e][:, None].astype(np.float64) * ye).astype(np.float32)
    return out.reshape(x.shape)


# revision 49
# speedup vs baseline: 1.0012x; 1.0012x over previous
"""MoE MLP (Mixtral-style top-2 routing) on 8 Trainium2 NeuronCores.

Strategy: expert-COLUMN-parallel (F-sharding). The router (tiny: T x H x E)
runs on host in fp32, exactly mirroring the reference math. Tokens are
grouped by expert on host; EVERY core processes EVERY expert's token group,
but only a 512-wide slice of the F dimension (core k owns F rows
[k*512,(k+1)*512) of each expert's Wg/Wu and the matching Wd columns).
Per-core work is therefore identical by construction — no load imbalance,
no padding to the max expert count — and each expert's weight slice is
loaded exactly once (48MB/core total). Each core produces a PARTIAL down
projection (its F-slice's contribution); the host sums the 8 partials in
f64 and applies the top-k combine weights in a weighted scatter-add.

Device layout (per core, everything feature-on-partition, token-on-free):
  hT   [H=1024, TC]  all experts' tokens, grouped+padded, transposed
  WgT  [H, 8*512]    per-expert F-slices of gate weight, transposed
  WuT  [H, 8*512]    up weight, same layout
  WdT  [8*512, H]    down weight slices, transposed
  yT   [H, TC]       partial expert outputs (to be summed across cores)

Program: one weight BLOCK per expert (double-buffered); each expert's
tokens form 1-2 passes (<=1152 resident tokens, h double-buffered via a
256-wide h_pre prefetch tile); per pass a loop over ct tiles. Gate/up
matmuls (f32r, full PE rate) accumulate over H in PSUM; ScalarE applies
silu; VectorE multiplies by the up projection; down matmuls accumulate the
512-wide F-slice in PSUM; VectorE copies to y_sb and gpsimd streams each
finished ct tile back to DRAM. The PE stream runs one ct-tile ahead of the
silu/mul stage to hide their latency.
"""

import numpy as np
import concourse.bass as bass
import concourse.mybir as mybir
from concourse.bass_utils import run_bass_kernel_spmd

f32 = mybir.dt.float32
f32r = mybir.dt.float32r

B, S, H, F, E = 4, 2048, 1024, 4096, 8
ACT_FUNC = mybir.ActivationFunctionType.Silu  # swapped in coresim_check.py
KT = H // 128  # 8 k-tiles of the H contraction
FSH = F // 8  # 512-wide per-core F slice
FT_PER = FSH // 128  # 4 f-tiles per slice
HT = H // 128  # 8 output H tiles
CT_W = 512  # max token tile width (moving dim N)
PASS_MAX = 1152  # SBUF budget for h_sb/y_sb columns


def _split_tiles(pass_size):
    """Split a pass into ct tiles, leading with a 256 tile (small first
    DMA so the PE can start early / h_pre prefetch is cheap), then greedy
    512s with the remainder pushed to the end (so the final tile — and the
    final y store — is as small as possible). All multiples of 128, >=256."""
    widths = [256]
    rest = pass_size - 256
    if rest < 256:
        assert 256 <= pass_size <= 512, pass_size
        return [pass_size]
    while rest:
        if rest <= 512:
            widths.append(rest)
            rest = 0
        elif rest - 512 >= 256:
            widths.append(512)
            rest -= 512
        else:
            widths.append(rest - 256)
            rest = 256
    assert sum(widths) == pass_size and all(256 <= w <= 512 for w in widths), widths
    return widths


def _expert_passes(ce_pad):
    """Split one expert's padded token count into passes of <=PASS_MAX,
    near-equal, multiples of 64."""
    n = -(-ce_pad // PASS_MAX)
    base = (ce_pad // n) // 64 * 64
    out = [base] * n
    rem = (ce_pad - base * n) // 64
    for i in range(rem):
        out[i] += 64
    assert sum(out) == ce_pad and all(256 <= ps <= PASS_MAX for ps in out), out
    return out


def build_program(expert_sizes, repeat=1):
    """Per-core Bass program. `expert_sizes`: padded token count per
    present expert (in block order). Each expert is one weight block
    spanning 1+ token passes. `repeat` re-runs everything (bench only)."""
    NWB = len(expert_sizes)
    pass_sizes = []
    pass_wb = []  # weight block (expert slot) per pass
    for b, ce in enumerate(expert_sizes):
        for ps in _expert_passes(ce):
            pass_sizes.append(ps)
            pass_wb.append(b)
    TC = sum(pass_sizes)
    tok0 = [sum(pass_sizes[:p]) for p in range(len(pass_sizes))]

    pass_sizes = pass_sizes * repeat
    pass_tok0 = tok0 * repeat
    pass_wb = [b + r * NWB for r in range(repeat) for b in pass_wb]
    NP = len(pass_sizes)
    NB = NWB * repeat  # global weight-block sequence length
    PSMAX = max(pass_sizes)
    tiles = [_split_tiles(ps) for ps in pass_sizes]
    NCT = [len(t) for t in tiles]
    tile_offs = [[sum(tiles[p][:i]) for i in range(NCT[p])] for p in range(NP)]

    # ctg enumeration: for p, for ct -> (p, ct, width, offset)
    ctg_base = [0] * (NP + 1)
    for p in range(NP):
        ctg_base[p + 1] = ctg_base[p] + NCT[p]
    TOTAL_CT = ctg_base[NP]
    ctg_pfc = []
    for p in range(NP):
        for ct in range(NCT[p]):
            ctg_pfc.append((p, ct, tiles[p][ct], tile_offs[p][ct]))

    # last ctg (exclusive) of each weight block
    blk_pass_last = {}
    for p in range(NP):
        blk_pass_last[pass_wb[p]] = p
    blk_ctg_end = {b: ctg_base[blk_pass_last[b] + 1] for b in blk_pass_last}
    blk_pass_first = {}
    for p in range(NP - 1, -1, -1):
        blk_pass_first[pass_wb[p]] = p

    SLOTS = max(NCT)

    # Per-tile-SLOT h DMA counts: slot j of pass p has been loaded
    # ht_cnt[p][j] times through pass p. DMA completions within a queue
    # burst are unordered, so each slot gets its OWN semaphore and waits
    # are on full per-slot cumulative totals (race-detector-valid: pass
    # p's slot-j load is gated on a gu that observed pass p-1's count).
    ht_cnt = []
    cnt = [0] * SLOTS
    for p in range(NP):
        for j in range(NCT[p]):
            cnt[j] += 1
        ht_cnt.append(list(cnt))

    # y store counts per slot, same scheme (stores are gated on yupds that
    # observed the previous pass's store counts)
    yd_cnt = []
    cnt = [0] * SLOTS
    for p in range(NP):
        for j in range(NCT[p]):
            cnt[j] += 4 if p == NP - 1 else 1
        yd_cnt.append(list(cnt))

    def _overlaps(p, lo, hi):
        """Tile indices of pass p whose column range intersects [lo, hi)."""
        return [
            i
            for i, (o, w) in enumerate(zip(tile_offs[p], tiles[p]))
            if o < hi and o + w > lo
        ]

    # Weight-block thresholds: block 0 is ft-granular on dedicated sems
    # (s_p[ft] pairs + s_wd); blocks >= 1 alternate parity sems s_w0/s_w1
    # (+48 each). The b-2 double-buffer gate proves all same-parity
    # predecessors complete, so full-block cumulative totals on the parity
    # sem are unambiguous even with blocks b-1 and b in flight.
    def swp_need(bs):
        assert bs >= 1
        n_parity = (bs + 1) // 2 if bs % 2 == 1 else bs // 2
        return 48 * n_parity

    nc = bass.Bass()
    hT = nc.declare_dram_parameter("hT", [H, TC], f32r, isOutput=False)
    wg = nc.declare_dram_parameter("WgT", [H, NWB * FSH], f32r, isOutput=False)
    wu = nc.declare_dram_parameter("WuT", [H, NWB * FSH], f32r, isOutput=False)
    wd = nc.declare_dram_parameter("WdT", [NWB * FSH, H], f32r, isOutput=False)
    yT = nc.declare_dram_parameter("yT", [H, TC], f32, isOutput=True)

    hT_v = hT.rearrange("(k p) t -> p k t", p=128)  # [128, KT, TC]
    wg_v = wg.rearrange("(k p) f -> p k f", p=128)  # [128, KT, NWB*FSH]
    wu_v = wu.rearrange("(k p) f -> p k f", p=128)
    wd_v = wd.rearrange("(q p) h -> p q h", p=128)  # [128, NWB*FT_PER, H]
    yT_v = yT.rearrange("(k p) t -> p k t", p=128)  # [128, HT, TC]

    from contextlib import ExitStack

    with ExitStack() as ctx:
        en = ctx.enter_context
        h_sb = en(nc.sbuf_tensor("h_sb", [128, KT, PSMAX], f32r))
        h_pre = en(nc.sbuf_tensor("h_pre", [128, KT, CT_W], f32r))
        y_sb = en(nc.sbuf_tensor("y_sb", [128, HT, PSMAX], f32))
        wg_sb = en(nc.sbuf_tensor("wg_sb", [128, 2, KT, FSH], f32r))
        wu_sb = en(nc.sbuf_tensor("wu_sb", [128, 2, KT, FSH], f32r))
        wd_sb = en(nc.sbuf_tensor("wd_sb", [128, 2, FT_PER, H], f32r))
        act_sb = en(nc.sbuf_tensor("act_sb", [128, 2, FT_PER, CT_W], f32r))

        g_ps = [en(nc.psum_tensor(f"g_ps{i}", [128, CT_W], f32)) for i in range(2)]
        u_ps = [en(nc.psum_tensor(f"u_ps{i}", [128, CT_W], f32)) for i in range(2)]
        yp_ps = [en(nc.psum_tensor(f"yp_ps{i}", [128, CT_W], f32)) for i in range(4)]

        s_h0 = en(nc.semaphore(name="s_h0"))  # pass-0 ct0 h (sync queue)
        s_pg = [en(nc.semaphore(name=f"s_pg{i}")) for i in range(FT_PER)]  # blk0 wg pieces
        s_pu = [en(nc.semaphore(name=f"s_pu{i}")) for i in range(FT_PER)]  # blk0 wu pieces
        s_wd0 = en(nc.semaphore(name="s_wd0"))  # blk0 wd
        s_w0 = en(nc.semaphore(name="s_w0"))  # even blocks >= 2 (48/blk)
        s_w1 = en(nc.semaphore(name="s_w1"))  # odd blocks (48/blk)
        s_ht = [en(nc.semaphore(name=f"s_ht{j}")) for j in range(SLOTS)]  # h tile slots
        s_yd = [en(nc.semaphore(name=f"s_yd{j}")) for j in range(SLOTS)]  # y store slots
        s_g = en(nc.semaphore(name="s_g"))  # PE: gate groups done (1/gi)
        s_u = en(nc.semaphore(name="s_u"))  # PE: up groups done (1/gi)
        s_silu = en(nc.semaphore(name="s_silu"))  # ACT: silu into act done (1/gi)
        s_mul = en(nc.semaphore(name="s_mul"))  # DVE: act *= up done (1/gi)
        s_down = en(nc.semaphore(name="s_down"))  # PE: down groups done (1/di)
        s_yupd = en(nc.semaphore(name="s_yupd"))  # DVE: y copy done (1/di)

        block = en(nc.Block())

        # ---------------- weight DMA stream (sync engine / HWDGE) --------
        @block.sync
        def _(sync):
            for bs in range(NB):
                b = bs % NWB  # slice index into the weight buffers
                buf = bs % 2
                if bs == 1:
                    # block 1 isn't needed until its first pass (~2 passes
                    # in); keep its 6MB out of the contended startup window
                    sync.wait_ge(s_h0, 16)
                    for j in range(1, NCT[0]):
                        sync.wait_ge(s_ht[j], 16 * ht_cnt[0][j])
                if bs >= 2:
                    # WAR: buffer bs%2 still read by block bs-2's gus/downs
                    sync.wait_ge(s_down, 8 * blk_ctg_end[bs - 2])
                fsl = slice(b * FSH, (b + 1) * FSH)
                qsl = slice(b * FT_PER, (b + 1) * FT_PER)
                if bs == 0:
                    # h ct0 rides the HWDGE queue ahead of the weight
                    # pieces — SWDGE's ~2us launch latency would otherwise
                    # land it AFTER wu0 and delay the first matmul
                    w0 = tiles[0][0]
                    sync.dma_start(h_sb[:, :, :w0], hT_v[:, :, :w0]).then_inc(
                        s_h0, 16
                    )
                    for ft in range(FT_PER):
                        f0 = b * FSH + ft * 128
                        sync.dma_start(
                            wg_sb[:, buf, :, ft * 128 : (ft + 1) * 128],
                            wg_v[:, :, f0 : f0 + 128],
                        ).then_inc(s_pg[ft], 16)
                        sync.dma_start(
                            wu_sb[:, buf, :, ft * 128 : (ft + 1) * 128],
                            wu_v[:, :, f0 : f0 + 128],
                        ).then_inc(s_pu[ft], 16)
                    if NCT[0] >= 2:
                        # wd isn't needed until the first down (~2 gu
                        # groups in); let h ct1 take the DMA engines first
                        sync.wait_ge(s_ht[1], 16)
                    sync.dma_start(wd_sb[:, buf], wd_v[:, qsl, :]).then_inc(s_wd0, 16)
                else:
                    sw = s_w1 if bs % 2 == 1 else s_w0
                    sync.dma_start(wg_sb[:, buf], wg_v[:, :, fsl]).then_inc(sw, 16)
                    sync.dma_start(wu_sb[:, buf], wu_v[:, :, fsl]).then_inc(sw, 16)
                    sync.dma_start(wd_sb[:, buf], wd_v[:, qsl, :]).then_inc(sw, 16)

        # ---------------- hT loads + y stores (gpsimd / SWDGE) -----------
        @block.gpsimd
        def _(gp):
            def load_h(p):
                # tile 0 of pass p>=1 goes to the h_pre prefetch buffer,
                # issued as soon as pass p-1's first gu released it
                if p >= 1:
                    gp.wait_ge(s_u, 4 * (ctg_base[p - 1] + 1))
                    w0 = tiles[p][0]
                    tsl = slice(pass_tok0[p], pass_tok0[p] + w0)
                    gp.dma_start(h_pre[:, :, :w0], hT_v[:, :, tsl]).then_inc(
                        s_ht[0], 16
                    )
                off = 0
                for i, wdt in enumerate(tiles[p]):
                    if i == 0:
                        # tile 0: pass 0 loads via the sync queue, passes
                        # >= 1 via the h_pre branch above
                        off += wdt
                        continue
                    if p >= 1:
                        # WAR on h_sb cols [off, off+wdt): last readers are
                        # pass p-1's gus of the overlapping tiles (PE is
                        # in-order, so this also covers all earlier passes;
                        # empty overlap -> m=-1 waits out passes < p-1)
                        m = max(_overlaps(p - 1, off, off + wdt), default=-1)
                        gp.wait_ge(s_u, 4 * (ctg_base[p - 1] + m + 1))
                    tsl = slice(pass_tok0[p] + off, pass_tok0[p] + off + wdt)
                    gp.dma_start(
                        h_sb[:, :, off : off + wdt], hT_v[:, :, tsl]
                    ).then_inc(s_ht[i], 16)
                    off += wdt

            def store_y_tiles(p):
                # stream each finished ct tile out as soon as its yupds
                # land; the last pass goes out in ht-pair pieces so the
                # final stores pipeline with the trailing yupds
                for j in range(NCT[p]):
                    ctg = ctg_base[p] + j
                    coff = tile_offs[p][j]
                    ctw = tiles[p][j]
                    tsl = slice(pass_tok0[p] + coff, pass_tok0[p] + coff + ctw)
                    if p == NP - 1:
                        for hp in range(4):
                            gp.wait_ge(s_yupd, 8 * ctg + 2 * (hp + 1))
                            gp.dma_start(
                                yT_v[:, 2 * hp : 2 * hp + 2, tsl],
                                y_sb[:, 2 * hp : 2 * hp + 2, coff : coff + ctw],
                            ).then_inc(s_yd[j], 16)
                    else:
                        gp.wait_ge(s_yupd, 8 * (ctg + 1))
                        gp.dma_start(
                            yT_v[:, :, tsl], y_sb[:, :, coff : coff + ctw]
                        ).then_inc(s_yd[j], 16)

            load_h(0)
            if NP > 1:
                load_h(1)
            for p in range(2, NP):
                # stores(p-2) gate on pass p-2's yupds, which complete
                # before load_h(p)'s gates (pass p-1 gus) — emit first
                store_y_tiles(p - 2)
                load_h(p)
            for p in range(max(NP - 2, 0), NP):
                store_y_tiles(p)

        # ---------------- PE stream (one ct-tile lookahead) ----------------
        @block.tensor
        def _(te):
            def gu(ctg):
                p, ct, ctw, coff = ctg_pfc[ctg]
                bs = pass_wb[p]
                buf = bs % 2
                if p == 0 and ct == 0:
                    te.wait_ge(s_h0, 16)
                elif ct == 0:
                    te.wait_ge(s_ht[0], 16 * (ht_cnt[p][0] - 1))
                else:
                    te.wait_ge(s_ht[ct], 16 * ht_cnt[p][ct])
                if ct == 0 and bs > 0 and p == blk_pass_first[bs]:
                    te.wait_ge(s_w1 if bs % 2 == 1 else s_w0, swp_need(bs))
                use_pre = p >= 1 and ct == 0
                csl = slice(coff, coff + ctw)
                for ft in range(FT_PER):
                    gi = ctg * 4 + ft
                    gb = gi % 2
                    if p == 0 and ct == 0:
                        te.wait_ge(s_pg[ft], 16)
                    if gi >= 2:
                        te.wait_ge(s_silu, gi - 1)
                    for k in range(KT):
                        rhs = h_pre[:, k, :ctw] if use_pre else h_sb[:, k, csl]
                        mm = nc.tensor.matmul(
                            g_ps[gb][:, :ctw],
                            wg_sb[:, buf, k, ft * 128 : (ft + 1) * 128],
                            rhs,
                            start=(k == 0),
                            stop=(k == KT - 1),
                        )
                        if k == KT - 1:
                            mm.then_inc(s_g, 1)
                    if p == 0 and ct == 0:
                        te.wait_ge(s_pu[ft], 16)
                    if gi >= 2:
                        te.wait_ge(s_mul, gi - 1)
                    for k in range(KT):
                        rhs = h_pre[:, k, :ctw] if use_pre else h_sb[:, k, csl]
                        mm = nc.tensor.matmul(
                            u_ps[gb][:, :ctw],
                            wu_sb[:, buf, k, ft * 128 : (ft + 1) * 128],
                            rhs,
                            start=(k == 0),
                            stop=(k == KT - 1),
                        )
                        if k == KT - 1:
                            mm.then_inc(s_u, 1)

            def down(ctg):
                p, ct, ctw, coff = ctg_pfc[ctg]
                bs = pass_wb[p]
                buf = bs % 2
                ab = ctg % 2
                if ct == 0 and p == blk_pass_first[bs]:
                    if bs == 0:
                        te.wait_ge(s_wd0, 16)
                    else:
                        te.wait_ge(s_w1 if bs % 2 == 1 else s_w0, swp_need(bs))
                te.wait_ge(s_mul, 4 * (ctg + 1))
                for ht in range(HT):
                    di = ctg * 8 + ht
                    db = di % 4
                    if di >= 4:
                        te.wait_ge(s_yupd, di - 3)
                    for ft in range(FT_PER):
                        mm = nc.tensor.matmul(
                            yp_ps[db][:, :ctw],
                            wd_sb[:, buf, ft, ht * 128 : (ht + 1) * 128],
                            act_sb[:, ab, ft, :ctw],
                            start=(ft == 0),
                            stop=(ft == FT_PER - 1),
                        )
                        if ft == FT_PER - 1:
                            mm.then_inc(s_down, 1)

            gu(0)
            for ctg in range(TOTAL_CT):
                if ctg + 1 < TOTAL_CT:
                    gu(ctg + 1)
                down(ctg)

        # ---------------- ACT stream (silu into act tile) ------------------
        @block.scalar
        def _(sc):
            for ctg in range(TOTAL_CT):
                ab = ctg % 2
                ctw = ctg_pfc[ctg][2]
                for ft in range(FT_PER):
                    gi = ctg * 4 + ft
                    gb = gi % 2
                    if ft == 0 and ctg >= 2:
                        # WAR on act_sb[ab]: down mms of ctg-2 done
                        sc.wait_ge(s_down, 8 * (ctg - 1))
                    sc.wait_ge(s_g, gi + 1)
                    nc.scalar.activation(
                        act_sb[:, ab, ft, :ctw],
                        g_ps[gb][:, :ctw],
                        ACT_FUNC,
                    ).then_inc(s_silu, 1)

        # ---------------- DVE stream (mul + y copy) ------------------
        @block.vector
        def _(ve):
            def muls(ctg):
                ab = ctg % 2
                ctw = ctg_pfc[ctg][2]
                for ft in range(FT_PER):
                    gi = ctg * 4 + ft
                    gb = gi % 2
                    ve.wait_ge(s_silu, gi + 1)
                    ve.wait_ge(s_u, gi + 1)
                    nc.vector.tensor_mul(
                        act_sb[:, ab, ft, :ctw],
                        act_sb[:, ab, ft, :ctw],
                        u_ps[gb][:, :ctw],
                    ).then_inc(s_mul, 1)

            def yupd(ctg):
                p, ct, ctw, coff = ctg_pfc[ctg]
                csl = slice(coff, coff + ctw)
                for ht in range(HT):
                    di = ctg * 8 + ht
                    db = di % 4
                    ve.wait_ge(s_down, di + 1)
                    if ht == 0 and p > 0:
                        # WAR on y_sb cols: stores of ALL overlapping tiles
                        # through pass p-1 must have drained (yd_cnt is
                        # cumulative, so this covers earlier passes too);
                        # empty overlap -> wait out every slot
                        ov = _overlaps(p - 1, coff, coff + ctw) or range(SLOTS)
                        for i in ov:
                            ve.wait_ge(s_yd[i], 16 * yd_cnt[p - 1][i])
                    nc.vector.tensor_copy(
                        y_sb[:, ht, csl], yp_ps[db][:, :ctw]
                    ).then_inc(s_yupd, 1)

            muls(0)
            for ctg in range(TOTAL_CT):
                # mirror the PE stream's emission order exactly
                if ctg + 1 < TOTAL_CT:
                    muls(ctg + 1)
                yupd(ctg)

    return nc


# ----------------------------------------------------------------------------
# Host side
# ----------------------------------------------------------------------------


def _route(h, Wr, topk):
    """Exact fp32 replica of the reference router. Returns sel [T,k], w [T,k]."""
    logits = h @ Wr.T  # [T, E]
    logits = logits.astype(np.float32)
    m = logits.max(axis=-1, keepdims=True)
    e = np.exp(logits - m)
    p = e / e.sum(axis=-1, keepdims=True)
    sel = np.argsort(-p, axis=-1, kind="stable")[:, :topk]  # ties -> lower idx
    w = np.take_along_axis(p, sel, axis=-1)
    if topk != 1:
        w = w / w.sum(axis=-1, keepdims=True)
    return sel, w.astype(np.float32)


def kernel(x, Wr, Wg, Wu, Wd, topk):
    topk = int(topk)
    x = np.asarray(x, dtype=np.float32)
    Wr = np.asarray(Wr, dtype=np.float32)
    Wg = np.asarray(Wg, dtype=np.float32)
    Wu = np.asarray(Wu, dtype=np.float32)
    Wd = np.asarray(Wd, dtype=np.float32)

    T = x.shape[0] * x.shape[1]
    h = np.ascontiguousarray(x.reshape(T, H))

    sel, w = _route(h, Wr, topk)

    idx = [None] * E
    wts = [None] * E
    for e in range(E):
        tok, kk = np.nonzero(sel == e)
        idx[e] = tok
        wts[e] = w[tok, kk]
    counts = [len(i) for i in idx]

    present = [e for e in range(E) if counts[e] > 0]
    sizes = [max(256, ((counts[e] + 63) // 64) * 64) for e in present]
    TC = sum(sizes)
    tok0 = [sum(sizes[:i]) for i in range(len(sizes))]

    nc = build_program(sizes)

    # hT: all experts' tokens grouped and padded — identical on every core
    hTfull = h.T  # [H, T] view
    hT = np.zeros((H, TC), dtype=np.float32)
    for i, e in enumerate(present):
        hT[:, tok0[i] : tok0[i] + counts[e]] = hTfull[:, idx[e]]

    # per-core weight slices: core k owns F rows [k*FSH,(k+1)*FSH) of every
    # expert (transposed layouts, concatenated in block order)
    in_maps = []
    for k in range(E):
        fs = slice(k * FSH, (k + 1) * FSH)
        WgT = np.empty((H, len(present) * FSH), dtype=np.float32)
        WuT = np.empty((H, len(present) * FSH), dtype=np.float32)
        WdT = np.empty((len(present) * FSH, H), dtype=np.float32)
        for i, e in enumerate(present):
            WgT[:, i * FSH : (i + 1) * FSH] = Wg[e, fs, :].T
            WuT[:, i * FSH : (i + 1) * FSH] = Wu[e, fs, :].T
            WdT[i * FSH : (i + 1) * FSH, :] = Wd[e, :, fs].T
        in_maps.append({"hT": hT, "WgT": WgT, "WuT": WuT, "WdT": WdT})

    res = run_bass_kernel_spmd(nc, in_maps, core_ids=list(range(E)))

    # sum the 8 partial projections in f64, then combine
    ysum = res.results[0]["yT"].astype(np.float64)
    for k in range(1, E):
        ysum += res.results[k]["yT"]
    out = np.zeros((T, H), dtype=np.float32)
    for i, e in enumerate(present):
        cnt = counts[e]
        ye = ysum[:, tok0[i] : tok0[i] + cnt].T  # [cnt, H] f64
        out[idx[e]] += (wts[e][:, None].astype(np.float64) * ye).astype(np.float32)
    return out.reshape(x.shape)
